# revision 70
# baseline (speedup 1.0000x reference)
"""DeepseekV2 decoder layer (MLA attention + SwiGLU MLP) on 8 TRN2 NeuronCores.

Sharding: core c -> batch b = c//4, query rows [j*512, (j+1)*512) with j = c%4.
Every core computes the full-sequence KV latents for its batch (cheap shared
latents, exactly MLA's design), its own 512 query rows through attention +
o_proj + FFN, and returns its 512 output rows. No collectives.

All cores run one identical SPMD program; per-core position enters only
through input data (causal masks, sliced hidden/rope tables).

On-device layout is feature-major (activations transposed, features on
partitions): for y = x @ W the device computes y^T = matmul(lhsT=W_tile,
rhs=x^T_tile) accumulating K-tiles in PSUM. RMSNorm weights are folded into
adjacent weight matrices on the host; RMSNorm 1/rms column scales are applied
when copying matmul outputs from PSUM to SBUF; cross-partition reductions use
ones-vector matmuls.

Keys are processed in per-core ROTATED chunk order: key block b holds the
original 512-column chunk (j+b)%4, so block 0 is the core's own query-aligned
chunk.  Block 0's hidden slice and ln1 stats are already resident from the q
path (no DMA, no stats), the causal mask becomes a shared diagonal band for
block 0 plus a constant 0/-50 exp-bias per key tile for the other blocks, and
the kv phase starts with zero load latency.  Stat/normalize chains are
software-pipelined so the in-order PE never waits on scalar/vector results;
softmax denominators pre-reduce groups of 4 exp tiles on the vector engine.
"""

import json

import numpy as np
import ml_dtypes

B, S, H = 2, 2048, 2048
NH = 16
Q_LORA = 1536
KV_LORA = 512
NOPE = 128
ROPE = 64
QHD = NOPE + ROPE  # 192
VHD = 128
FF = 8192
EPS = 1e-6
P = 128
QR = 512  # query rows per core
TK = S // P  # 16 key tiles
TQ = QR // P  # 4
KI_H = H // P  # 16
KI_QL = Q_LORA // P  # 12
KI_KVL = KV_LORA // P  # 4
NF_FF = FF // P  # 64
NBLK = S // QR  # 4 key blocks
ATTN_SCALE = QHD ** -0.5
MASK_NEG = -50.0

BF16 = ml_dtypes.bfloat16

_COMPILED = {}


# ---------------------------------------------------------------------------
# compiler workaround: this container's walrus rejects >1 sem wait per
# instruction; split extra waits onto single-wait NoOps.
# ---------------------------------------------------------------------------
def _install_multiwait_fix(bass):
    if getattr(bass.Bass, "_multiwait_fix_installed", False):
        return
    orig = bass.Bass.to_json_bytes

    def _split(m):
        for f in m.get("functions", []):
            for b in f.get("blocks", []):
                out = []
                for inst in b.get("instructions", []):
                    si = inst.get("sync_info") or {}
                    waits = si.get("on_wait") or []
                    if len(waits) > 1:
                        for k, w in enumerate(waits[:-1]):
                            out.append(
                                {
                                    "debug": inst.get("debug", 0),
                                    "engine": inst["engine"],
                                    "ins": [],
                                    "name": f"{inst['name']}_w{k}",
                                    "opcode": "NoOp",
                                    "outs": [],
                                    "sync_info": {"on_update": [], "on_wait": [w]},
                                }
                            )
                        si["on_wait"] = [waits[-1]]
                    out.append(inst)
                b["instructions"] = out
        return m

    def patched(self):
        raw = orig(self)
        try:
            return json.dumps(_split(json.loads(raw))).encode()
        except Exception:
            return raw

    bass.Bass.to_json_bytes = patched
    bass.Bass._multiwait_fix_installed = True


def _install_drain_fix(tile, ScopedClock, VectorClock):
    if getattr(tile.TileContext, "_drain_fix_installed", False):
        return

    def _drain_and_barrier(self, tick_clock, wait_clock):
        gc = tick_clock.global_clock
        n = len(gc)
        for p in range(n):
            t = gc[p]
            if t > 0:
                vc = VectorClock([0] * n)
                vc.require_at_least(p, t)
                d = self.nc.sync.drain()
                wait_clock.add_sem_waits(d.ins, ScopedClock({None: vc}))
        self.nc.all_engine_barrier()
        popped = self.nc._tile_sem_poison_stack.pop()
        assert popped is self._sem_poison
        self.nc.clear_and_free_semaphores(list(self.sems.allocated().values()))
        self.nc.all_engine_barrier()

    tile.TileContext._drain_and_barrier = _drain_and_barrier
    tile.TileContext._drain_fix_installed = True


# ---------------------------------------------------------------------------
# device program
# ---------------------------------------------------------------------------
def _build_nc():
    import concourse.bass as bass
    import concourse.mybir as mybir
    import concourse.tile as tile
    from concourse.vector_clock import ScopedClock, VectorClock

    _install_multiwait_fix(bass)
    _install_drain_fix(tile, ScopedClock, VectorClock)

    dt = mybir.dt
    AF = mybir.ActivationFunctionType
    MUL = mybir.AluOpType.mult
    ADD = mybir.AluOpType.add
    SUB = mybir.AluOpType.subtract

    nc = bass.Bass()

    # register EPS so float bias=EPS works on the scalar engine
    _eps_t = nc.alloc_sbuf_tensor(f"const-float32-{EPS}", [128, 1], dt.float32)
    nc.gpsimd.memset(_eps_t.ap(), EPS)
    nc.const_aps.aps[(dt.float32, EPS)] = _eps_t.ap()
    nc.all_engine_barrier()

    # ---- inputs ----
    # hTb holds key blocks 1..3 only (block 0 == the resident q slice)
    hTb = nc.dram_tensor("hTb", [H, S - QR], dt.bfloat16, kind="ExternalInput")
    hTqb = nc.dram_tensor("hTqb", [H, QR], dt.bfloat16, kind="ExternalInput")
    hTq = nc.dram_tensor("hTq", [H, QR], dt.float32, kind="ExternalInput")
    # rope tables are [64] rows: cos duplicated, sin with the rotate-half
    # signs baked in ([-sin; sin]) so rope runs as 3 full-width DVE ops
    cosT = nc.dram_tensor("cosT", [ROPE, S], dt.bfloat16, kind="ExternalInput")
    sinT = nc.dram_tensor("sinT", [ROPE, S], dt.bfloat16, kind="ExternalInput")
    cosTq = nc.dram_tensor("cosTq", [ROPE, QR], dt.float32, kind="ExternalInput")
    sinTq = nc.dram_tensor("sinTq", [ROPE, QR], dt.float32, kind="ExternalInput")
    maskdiag = nc.dram_tensor("maskdiag", [P, TQ, QR], dt.bfloat16, kind="ExternalInput")
    colmask = nc.dram_tensor("colmask", [P, TK], dt.float32, kind="ExternalInput")
    w_qa = nc.dram_tensor("w_qa", [KI_QL, P, KI_H // 2, 2, P], dt.float8e4, kind="ExternalInput")
    w_qb = nc.dram_tensor("w_qb", [NH, P, KI_QL // 2, 2, QHD], dt.float8e4, kind="ExternalInput")
    w_kva = nc.dram_tensor("w_kva", [P, KI_H, KV_LORA + ROPE], dt.bfloat16, kind="ExternalInput")
    w_kv_k = nc.dram_tensor("w_kv_k", [NH // 4, P, KI_KVL // 2, 2, 512], dt.float8e4, kind="ExternalInput")
    w_kv_v = nc.dram_tensor("w_kv_v", [NH // 4, P, KI_KVL, 512], dt.bfloat16, kind="ExternalInput")
    w_o = nc.dram_tensor("w_o", [KI_H, P, NH, VHD], dt.bfloat16, kind="ExternalInput")
    w_g = nc.dram_tensor("w_g", [NF_FF, P, KI_H, P], dt.bfloat16, kind="ExternalInput")
    w_u = nc.dram_tensor("w_u", [NF_FF, P, KI_H, P], dt.bfloat16, kind="ExternalInput")
    w_d = nc.dram_tensor("w_d", [KI_H, P, NF_FF, P], dt.bfloat16, kind="ExternalInput")
    out = nc.dram_tensor("out", [H, QR], dt.float32, kind="ExternalOutput")

    def rsqrt_stat(tmp, acc, denom):
        # 1/sqrt(mean + eps) from a [1, N] PSUM sum-of-squares accumulator
        s = tmp.tile([1, acc.shape[-1]], dt.float32, tag="stat", bufs=2)
        nc.scalar.activation(out=s[:], in_=acc[:], func=AF.Sqrt, bias=EPS, scale=1.0 / denom)
        nc.vector.reciprocal(s[:], s[:])
        return s

    import contextlib

    with tile.TileContext(nc) as tc, contextlib.ExitStack() as top:
        tp = lambda **kw: top.enter_context(tc.tile_pool(**kw))
        ones = tp(name="ones", bufs=1)
        tmp = tp(name="tmp", bufs=3)
        ld = tp(name="ld", bufs=3)
        ps = tp(name="ps", bufs=3, space="PSUM")
        ps_acc = tp(name="ps_acc", bufs=1, space="PSUM")
        # prefetch tiles; their loads are issued just after the xqbf loads
        # (first-needed data first in the DMA queues)
        pre = tp(name="pre", bufs=1)
        wkh0 = pre.tile([P, KI_KVL // 2, 2, 512], dt.float8e4)
        wvh0 = pre.tile([P, KI_KVL, 512], dt.bfloat16)
        wkva = pre.tile([P, KI_H, KV_LORA + ROPE], dt.bfloat16)

        def issue_prefetch():
            nc.sync.dma_start(wkva[:], w_kva[:])
            nc.sync.dma_start(wkh0[:], w_kv_k[0])
            nc.sync.dma_start(wvh0[:], w_kv_v[0])
        # h1 (residual after attention) stays resident in SBUF
        h1_pool = tp(name="h1_pool", bufs=1)
        h1sb = h1_pool.tile([P, KI_H, QR], dt.bfloat16)
        s2rep = h1_pool.tile([P, QR], dt.float32)

        # [1, P] row for partition replication (K=1 matmul),
        # [P, 1] column for cross-partition reduction (M=1 matmul).
        ones_f32 = ones.tile([1, P], dt.float32)
        nc.vector.memset(ones_f32[:], 1.0)
        ones_bf = ones.tile([P, 1], dt.bfloat16)
        nc.vector.memset(ones_bf[:], 1.0)

        with contextlib.ExitStack() as mid:
            lat = mid.enter_context(tc.tile_pool(name="lat", bufs=1))
            ckv = lat.tile([P, KI_KVL, S], dt.bfloat16)  # normalized kv latents
            # fp8 copy of the latents packed for DoubleRow k_nope builds
            # (v projections keep the bf16 ckv for accuracy)
            ckv8 = lat.tile([P, 2, KI_KVL // 2, S], dt.float8e4)
            kpe = lat.tile([ROPE, S], dt.float8e4)  # roped shared 16*key-pe
            mdg = lat.tile([P, TQ, QR], dt.bfloat16)  # block-0 diagonal mask
            cmask = lat.tile([P, TK], dt.float32)  # 1/0 mask per key tile
            attn_pool = mid.enter_context(tc.tile_pool(name="attn_pool", bufs=1))
            attn = attn_pool.tile([P, NH, QR], dt.bfloat16)
            qnp = mid.enter_context(tc.tile_pool(name="qnp", bufs=1))
            # q packed for DoubleRow fp8 scores: slot 0 = 16*q_nope,
            # slot 1 = 16*q_pe on partitions 0-63, zero pad on 64-127
            qpk = qnp.tile([P, 2, NH, QR], dt.float8e4)
            nc.gpsimd.memset(qpk[ROPE:, 1, :, :], 0.0)

            with contextlib.ExitStack() as scA:
                pA = scA.enter_context(tc.tile_pool(name="pA", bufs=1))
                xqbf = pA.tile([P, KI_H, QR], dt.bfloat16)
                s1qrep = pA.tile([P, QR], dt.float32)

                # ==== phase 1: q path (kv-block loads stream in behind) ====
                accq = ps_acc.tile([1, QR], dt.float32, tag="acc")
                for ki in range(KI_H):
                    nc.sync.dma_start(xqbf[:, ki, :], hTqb[ki * P : (ki + 1) * P, :])
                    sq = tmp.tile([P, QR], dt.bfloat16, tag="sq")
                    nc.vector.tensor_tensor(sq[:], xqbf[:, ki, :], xqbf[:, ki, :], MUL)
                    nc.tensor.matmul(
                        accq[:], ones_bf[:], sq[:], start=(ki == 0), stop=(ki == KI_H - 1)
                    )
                s1q = rsqrt_stat(tmp, accq, H)
                issue_prefetch()
                nc.sync.dma_start(mdg[:], maskdiag[:])
                nc.sync.dma_start(cmask[:], colmask[:])

                with tc.tile_pool(name="p2", bufs=1) as p2:
                    # 16*x packed fp8 for DoubleRow q_a (ln1 q stats and the
                    # kv block-0 projections keep using the bf16 xqbf)
                    xq8 = p2.tile([P, 2, KI_H // 2, QR], dt.float8e4)
                    for ki in range(KI_H):
                        nc.gpsimd.tensor_scalar_mul(
                            xq8[:, ki % 2, ki // 2, :], xqbf[:, ki, :], 16.0
                        )
                    # qlat8 = 16*qlat: psum is 256*(x@w_qa); (1/16)/rms scale
                    qlat = p2.tile([P, 2, KI_QL // 2, QR], dt.float8e4)
                    qacc = ps_acc.tile([1, QR], dt.float32, tag="acc")
                    prev_sq = None
                    for nf in range(KI_QL):
                        wt = p2.tile([P, KI_H // 2, 2, P], dt.float8e4, tag="wqa", bufs=2)
                        nc.sync.dma_start(wt[:], w_qa[nf])
                        pt = ps.tile([P, QR], dt.float32, tag="mm")
                        for g in range(KI_H // 2):
                            nc.tensor.matmul(
                                pt[:],
                                wt[:, g, :, :],
                                xq8[:, :, g, :],
                                start=(g == 0),
                                stop=(g == KI_H // 2 - 1),
                                perf_mode=mybir.MatmulPerfMode.DoubleRow,
                            )
                        if nf == 0:
                            # replicate 1/rms after the first q_a chain so the
                            # PE never waits on the sqrt/reciprocal latency
                            repq = ps.tile([P, QR], dt.float32, tag="mm")
                            nc.tensor.matmul(repq[:], ones_f32[:], s1q[:], start=True, stop=True)
                            nc.vector.tensor_copy(s1qrep[:], repq[:])
                        if prev_sq is not None:
                            nc.tensor.matmul(
                                qacc[:], ones_bf[:], prev_sq[:],
                                start=(nf == 1), stop=False,
                            )
                        nc.vector.scalar_tensor_tensor(
                            qlat[:, nf % 2, nf // 2, :], pt[:], 1.0 / 16.0,
                            s1qrep[:], MUL, MUL,
                        )
                        sq = tmp.tile([P, QR], dt.bfloat16, tag="sq")
                        nc.vector.tensor_tensor(
                            sq[:], qlat[:, nf % 2, nf // 2, :],
                            qlat[:, nf % 2, nf // 2, :], MUL,
                        )
                        prev_sq = sq

                    sqrep = p2.tile([P, QR], dt.float32)
                    cosq = p2.tile([ROPE, QR], dt.float32)
                    sinq = p2.tile([ROPE, QR], dt.float32)
                    nc.sync.dma_start(cosq[:], cosTq[:])
                    nc.sync.dma_start(sinq[:], sinTq[:])

                    def qb_mm(h):
                        wt = p2.tile([P, KI_QL // 2, 2, QHD], dt.float8e4, tag="wqb", bufs=2)
                        nc.sync.dma_start(wt[:], w_qb[h])
                        pt = ps.tile([P, QR], dt.float32, tag="mm")
                        for g in range(KI_QL // 2):
                            nc.tensor.matmul(
                                pt[:], wt[:, g, :, :NOPE], qlat[:, :, g, :],
                                start=(g == 0), stop=(g == KI_QL // 2 - 1),
                                perf_mode=mybir.MatmulPerfMode.DoubleRow,
                            )
                        if h == 0:
                            # finish the q_a_ln stat chain + replicate between
                            # head 0's two matmul chains (latency hiding).
                            # qacc holds 256*sum(qlat^2), so 1/sqrt(mean*256)
                            # = (1/16)/rms; psum q is 256x -> qpk = 16*q/rms
                            nc.tensor.matmul(
                                qacc[:], ones_bf[:], prev_sq[:], start=False, stop=True
                            )
                            sqv = rsqrt_stat(tmp, qacc, Q_LORA)
                        pt2 = ps.tile([ROPE, QR], dt.float32, tag="mm")
                        for g in range(KI_QL // 2):
                            nc.tensor.matmul(
                                pt2[:], wt[:, g, :, NOPE:QHD], qlat[:, :, g, :],
                                start=(g == 0), stop=(g == KI_QL // 2 - 1),
                                perf_mode=mybir.MatmulPerfMode.DoubleRow,
                            )
                        if h == 0:
                            repsq = ps.tile([P, QR], dt.float32, tag="mm")
                            nc.tensor.matmul(repsq[:], ones_f32[:], sqv[:], start=True, stop=True)
                            nc.vector.tensor_copy(sqrep[:], repsq[:])
                            # fold q_a_ln scale into the q rope tables
                            nc.vector.tensor_tensor(cosq[:], cosq[:], sqrep[:ROPE, :], MUL)
                            nc.vector.tensor_tensor(sinq[:], sinq[:], sqrep[:ROPE, :], MUL)
                        return pt, pt2

                    def qb_dve(h, pt, pt2):
                        nc.vector.tensor_tensor(qpk[:, 0, h, :], pt[:], sqrep[:], MUL)
                        pesq = p2.tile([ROPE, QR], dt.float32, tag="pes", bufs=2)
                        nc.vector.tensor_copy(pesq[:], pt2[:])
                        # swapped halves for rotate-half (signs live in sinq)
                        xsw = p2.tile([ROPE, QR], dt.float32, tag="x2h", bufs=2)
                        nc.sync.dma_start(xsw[:32, :], pesq[32:, :])
                        nc.sync.dma_start(xsw[32:, :], pesq[:32, :])
                        t1 = p2.tile([ROPE, QR], dt.float32, tag="t1", bufs=2)
                        t2 = p2.tile([ROPE, QR], dt.float32, tag="t2", bufs=2)
                        nc.vector.tensor_tensor(t1[:], pesq[:], cosq[:], MUL)
                        nc.vector.tensor_tensor(t2[:], xsw[:], sinq[:], MUL)
                        nc.vector.tensor_tensor(qpk[:ROPE, 1, h, :], t1[:], t2[:], ADD)

                    for h in range(NH):
                        pt, pt2 = qb_mm(h)
                        qb_dve(h, pt, pt2)

                # ==== phase 2: kv latents per 512-col key block ============
                # block 0 reuses xqbf + s1qrep (no loads, no ln1 stats); the
                # stat/normalize chains are software-pipelined across blocks.
                with tc.tile_pool(name="pB", bufs=1) as pB:

                    def wkva_sl(ki, csl):
                        return wkva[:, ki, csl]
                    cosb = pB.tile([ROPE, S], dt.bfloat16)
                    sinb = pB.tile([ROPE, S], dt.bfloat16)
                    nc.sync.dma_start(cosb[:], cosT[:])
                    nc.sync.dma_start(sinb[:], sinT[:])

                    xcs = {}
                    s1rs = {}
                    kvaccs = {}
                    pend_rep1 = {}
                    pend_repkv = {}

                    def kv_stats(t):
                        # ln1 sum-of-squares for loaded block t (PE chain)
                        xc = pB.tile([P, KI_H, 512], dt.bfloat16, tag="xc", bufs=2)
                        xcs[t] = xc
                        acc = ps_acc.tile([1, 512], dt.float32, tag="acc")
                        for ki in range(KI_H):
                            nc.sync.dma_start(
                                xc[:, ki, :], hTb[ki * P : (ki + 1) * P, (t - 1) * 512 : t * 512]
                            )
                            sq = tmp.tile([P, 512], dt.bfloat16, tag="sq")
                            nc.vector.tensor_tensor(sq[:], xc[:, ki, :], xc[:, ki, :], MUL)
                            nc.tensor.matmul(
                                acc[:], ones_bf[:], sq[:], start=(ki == 0), stop=(ki == KI_H - 1)
                            )
                        pend_rep1[t] = rsqrt_stat(tmp, acc, H)

                    def kv_rep1(t):
                        s1 = pend_rep1.pop(t)
                        rep1 = ps.tile([P, 512], dt.float32, tag="mm")
                        nc.tensor.matmul(rep1[:], ones_f32[:], s1[:], start=True, stop=True)
                        s1r = tmp.tile([P, 512], dt.float32, tag="s1r", bufs=2)
                        nc.vector.tensor_copy(s1r[:], rep1[:])
                        s1rs[t] = s1r

                    def kv_proj_mm(t, nf):
                        # one latent K-tile projection for block t (PE only)
                        rhs = xqbf if t == 0 else xcs[t]
                        pt = ps.tile([P, 512], dt.float32, tag="mm")
                        for ki in range(KI_H):
                            nc.tensor.matmul(
                                pt[:],
                                wkva_sl(ki, slice(nf * P, (nf + 1) * P)),
                                rhs[:, ki, :],
                                start=(ki == 0),
                                stop=(ki == KI_H - 1),
                            )
                        return pt

                    def kv_proj_dve(t, nf, pt):
                        # ln1 1/rms column scale on the way out of PSUM
                        tsl = slice(t * 512, (t + 1) * 512)
                        s1r = s1qrep if t == 0 else s1rs[t]
                        nc.vector.tensor_tensor(ckv[:, nf, tsl], pt[:], s1r[:], MUL)
                        sq = tmp.tile([P, 512], dt.bfloat16, tag="sq")
                        nc.vector.tensor_tensor(sq[:], ckv[:, nf, tsl], ckv[:, nf, tsl], MUL)
                        return sq

                    def kv_acc_mm(t, nf, sq):
                        if t not in kvaccs:
                            kvaccs[t] = ps_acc.tile(
                                [1, 512], dt.float32, tag="acc", name="kvacc"
                            )
                        nc.tensor.matmul(
                            kvaccs[t][:], ones_bf[:], sq[:],
                            start=(nf == 0), stop=(nf == KI_KVL - 1),
                        )
                        if nf == KI_KVL - 1:
                            pend_repkv[t] = rsqrt_stat(tmp, kvaccs.pop(t), KV_LORA)

                    def kv_rope(t):
                        tsl = slice(t * 512, (t + 1) * 512)
                        rhs = xqbf if t == 0 else xcs[t]
                        s1r = s1qrep if t == 0 else s1rs[t]
                        pt = ps.tile([ROPE, 512], dt.float32, tag="mm")
                        for ki in range(KI_H):
                            nc.tensor.matmul(
                                pt[:],
                                wkva_sl(ki, slice(KV_LORA, KV_LORA + ROPE)),
                                rhs[:, ki, :],
                                start=(ki == 0),
                                stop=(ki == KI_H - 1),
                            )
                        pes = pB.tile([ROPE, 512], dt.float32, tag="pes", bufs=1)
                        nc.vector.tensor_tensor(pes[:], pt[:], s1r[:ROPE, :], MUL)
                        # swapped halves for rotate-half (signs live in sinb)
                        xsw = pB.tile([ROPE, 512], dt.float32, tag="x2h", bufs=1)
                        nc.sync.dma_start(xsw[:32, :], pes[32:, :])
                        nc.sync.dma_start(xsw[32:, :], pes[:32, :])
                        t1 = pB.tile([ROPE, 512], dt.float32, tag="t1", bufs=1)
                        t2 = pB.tile([ROPE, 512], dt.float32, tag="t2", bufs=1)
                        nc.vector.tensor_tensor(t1[:], pes[:], cosb[:, tsl], MUL)
                        nc.vector.tensor_tensor(t2[:], xsw[:], sinb[:, tsl], MUL)
                        nc.vector.tensor_tensor(kpe[:, tsl], t1[:], t2[:], ADD)

                    def kv_scale(t):
                        # apply the kv_a rmsnorm 1/rms to block t's latents
                        skv = pend_repkv.pop(t)
                        repkv = ps.tile([P, 512], dt.float32, tag="mm")
                        nc.tensor.matmul(repkv[:], ones_f32[:], skv[:], start=True, stop=True)
                        rkv = tmp.tile([P, 512], dt.float32, tag="s1r", bufs=2)
                        nc.vector.tensor_copy(rkv[:], repkv[:])
                        tsl = slice(t * 512, (t + 1) * 512)
                        for nf in range(KI_KVL):
                            nc.vector.tensor_tensor(ckv[:, nf, tsl], ckv[:, nf, tsl], rkv[:], MUL)
                            nc.gpsimd.tensor_copy(ckv8[:, nf % 2, nf // 2, tsl], ckv[:, nf, tsl])

                    def kv_block_work(t):
                        # projections with the kvacc chain trailing one K-tile;
                        # for loaded blocks the 1/rms replicate slots in after
                        # the first projection chain (hides sqrt+recip latency)
                        sqs = []
                        for nf in range(KI_KVL):
                            pt = kv_proj_mm(t, nf)
                            if nf == 0 and t > 0:
                                kv_rep1(t)
                            sqs.append(kv_proj_dve(t, nf, pt))
                            if nf >= 1:
                                kv_acc_mm(t, nf - 1, sqs[nf - 1])
                        # kvacc (and its sqrt+reciprocal) ahead of the rope
                        # DVE chain, so the reciprocal isn't queued behind it
                        kv_acc_mm(t, KI_KVL - 1, sqs[-1])
                        kv_rope(t)

                    kv_block_work(0)
                    for t in range(1, NBLK):
                        kv_stats(t)
                        kv_scale(t - 1)
                        kv_block_work(t)
                    # kv_scale(3) is deferred into the attention phase

            # ==== phase 3: attention ====
            with tc.tile_pool(name="p3", bufs=1) as p3, tc.tile_pool(
                name="ps_att", bufs=1, space="PSUM"
            ) as ps_att:
                # previous head's softmax normalize: se is staged to SBUF on
                # the scalar engine, replicated by the PE (no slow input),
                # and the reciprocal runs FULL-WIDTH on the replicated
                # [128,512] tile (~270ns) instead of on [1,512] (~3.3us).
                pending = []  # (h, av, se_sb)

                def flush_norm():
                    while pending:
                        h_, av_, ses = pending.pop(0)
                        repr_ = ps.tile([P, QR], dt.float32, tag="mm")
                        nc.tensor.matmul(repr_[:], ones_f32[:], ses[:], start=True, stop=True)
                        rsb = tmp.tile([P, QR], dt.float32, tag="s1r", bufs=2)
                        nc.vector.reciprocal(rsb[:], repr_[:])
                        nc.vector.tensor_tensor(attn[:, h_, :], av_[:], rsb[:], MUL)

                for hg in range(NH // 4):
                    if hg == 0:
                        wkh, wvh = wkh0, wvh0
                    else:
                        wkh = p3.tile([P, KI_KVL // 2, 2, 512], dt.float8e4, tag="wkh", bufs=2)
                        nc.sync.dma_start(wkh[:], w_kv_k[hg])
                        wvh = p3.tile([P, KI_KVL, 512], dt.bfloat16, tag="wvh", bufs=2)
                        nc.sync.dma_start(wvh[:], w_kv_v[hg])
                    # v for 4 heads at once: v_rm[kpos, 4*VHD]
                    vsb = p3.tile([P, TK, 4 * VHD], dt.bfloat16, tag="vsb", bufs=2)
                    for kt in range(TK):
                        if hg == 0 and kt == 12:
                            # blocks 0-2 are done; finish block 3's kv norm
                            # while the PE is busy with the first 12 tiles
                            kv_scale(3)
                        pt = ps.tile([P, 4 * VHD], dt.float32, tag="mm")
                        for lt in range(KI_KVL):
                            nc.tensor.matmul(
                                pt[:],
                                ckv[:, lt, kt * P : (kt + 1) * P],
                                wvh[:, lt, :],
                                start=(lt == 0),
                                stop=(lt == KI_KVL - 1),
                            )
                        nc.vector.tensor_copy(vsb[:, kt, :], pt[:])
                    for hh in range(4):
                        h = hg * 4 + hh
                        # k packed for DoubleRow fp8: slot 0 = 16*k_nope
                        # (host-folded into w_kv_k), slot 1 = 16*k_pe + pad
                        ksb = p3.tile([P, 2, S], dt.float8e4, tag="ksb", bufs=2)
                        nc.gpsimd.memset(ksb[ROPE:, 1, :], 0.0)
                        for t in range(NBLK):
                            pt = ps.tile([P, 512], dt.float32, tag="mm")
                            for g in range(KI_KVL // 2):
                                nc.tensor.matmul(
                                    pt[:],
                                    wkh[:, g, :, hh * P : (hh + 1) * P],
                                    ckv8[:, :, g, t * 512 : (t + 1) * 512],
                                    start=(g == 0),
                                    stop=(g == KI_KVL // 2 - 1),
                                    perf_mode=mybir.MatmulPerfMode.DoubleRow,
                                )
                            tsl = slice(t * 512, (t + 1) * 512)
                            nc.vector.tensor_copy(ksb[:, 0, tsl], pt[:])
                            nc.gpsimd.tensor_copy(ksb[:ROPE, 1, tsl], kpe[:, tsl])
                        # scores / masked exp / attnV over all key tiles
                        av = ps_att.tile([P, QR], dt.float32, tag="av", bufs=2)
                        se = ps_att.tile([1, QR], dt.float32, tag="se", bufs=2)
                        # 2-deep software pipeline: emit av for kt-2 so the
                        # PE never stalls on the exp+mask chain.  The softmax
                        # denominator pre-reduces groups of GS exp tiles on
                        # the vector engine (4x fewer M=1 PE matmuls).
                        DELAY = 2
                        GS = 4
                        prs = {}

                        def _drain_kt(kt):
                            pr4, slot = prs.pop(kt)
                            nc.tensor.matmul(
                                av[:], vsb[:, kt, hh * VHD : (hh + 1) * VHD],
                                pr4[:, slot, :],
                                start=(kt == 0), stop=(kt == TK - 1),
                            )

                        pr4 = None
                        for kt in range(TK):
                            if kt == 4:
                                flush_norm()
                            g, slot = divmod(kt, GS)
                            if slot == 0:
                                pr4 = p3.tile([P, GS, QR], dt.bfloat16, tag="pr4", bufs=2)
                            sc = ps.tile([P, QR], dt.float32, tag="mm")
                            nc.tensor.matmul(
                                sc[:], ksb[:, :, kt * P : (kt + 1) * P],
                                qpk[:, :, h, :],
                                start=True, stop=True,
                                perf_mode=mybir.MatmulPerfMode.DoubleRow,
                            )
                            # scores carry 16*16 = 256x from the fp8 scaling
                            nc.scalar.activation(
                                out=pr4[:, slot, :], in_=sc[:], func=AF.Exp,
                                scale=ATTN_SCALE / 256.0,
                            )
                            if kt < TQ:
                                # block 0 = the diagonal block: banded mask
                                nc.vector.tensor_tensor(
                                    pr4[:, slot, :], pr4[:, slot, :], mdg[:, kt, :], MUL
                                )
                            else:
                                # other blocks all-past (1.0) or all-future
                                # (0.0); per-key-tile scalar on idle GPSIMD
                                nc.gpsimd.tensor_scalar_mul(
                                    pr4[:, slot, :], pr4[:, slot, :],
                                    cmask[:, kt : kt + 1],
                                )
                            prs[kt] = (pr4, slot)
                            if kt >= DELAY:
                                _drain_kt(kt - DELAY)
                            if slot == GS - 1:
                                t01 = p3.tile([P, QR], dt.bfloat16, tag="t01", bufs=2)
                                t23 = p3.tile([P, QR], dt.bfloat16, tag="t23", bufs=2)
                                gsum = p3.tile([P, QR], dt.bfloat16, tag="gsum", bufs=2)
                                nc.vector.tensor_tensor(t01[:], pr4[:, 0, :], pr4[:, 1, :], ADD)
                                nc.vector.tensor_tensor(t23[:], pr4[:, 2, :], pr4[:, 3, :], ADD)
                                nc.vector.tensor_tensor(gsum[:], t01[:], t23[:], ADD)
                                nc.tensor.matmul(
                                    se[:], ones_bf[:], gsum[:],
                                    start=(g == 0), stop=(g == TK // GS - 1),
                                )
                        for kt in range(TK - DELAY, TK):
                            _drain_kt(kt)
                        se_sb = tmp.tile([1, QR], dt.float32, tag="stat", bufs=2)
                        nc.scalar.activation(out=se_sb[:], in_=se[:], func=AF.Copy)
                        pending.append((h, av, se_sb))
                flush_norm()

            # ==== phase 4: o_proj + residual + ln2 (h1 resident in SBUF) ====
            with tc.tile_pool(name="p4", bufs=1) as p4:
                oacc = ps_acc.tile([1, QR], dt.float32, tag="acc")
                for nf in range(KI_H):
                    wt = p4.tile([P, NH, VHD], dt.bfloat16, tag="wo", bufs=2)
                    nc.sync.dma_start(wt[:], w_o[nf])
                    pt = ps.tile([P, QR], dt.float32, tag="mm")
                    for kh in range(NH):
                        nc.tensor.matmul(
                            pt[:],
                            wt[:, kh, :],
                            attn[:, kh, :],
                            start=(kh == 0),
                            stop=(kh == NH - 1),
                        )
                    ht = ld.tile([P, QR], dt.float32, tag="hload")
                    nc.sync.dma_start(ht[:], hTq[nf * P : (nf + 1) * P, :])
                    nc.vector.tensor_tensor(h1sb[:, nf, :], pt[:], ht[:], ADD)
                    sq = tmp.tile([P, QR], dt.bfloat16, tag="sq")
                    nc.vector.tensor_tensor(sq[:], h1sb[:, nf, :], h1sb[:, nf, :], MUL)
                    nc.tensor.matmul(
                        oacc[:], ones_bf[:], sq[:], start=(nf == 0), stop=(nf == KI_H - 1)
                    )
                s2 = rsqrt_stat(tmp, oacc, H)
                reps2 = ps.tile([P, QR], dt.float32, tag="mm")
                nc.tensor.matmul(reps2[:], ones_f32[:], s2[:], start=True, stop=True)
                nc.vector.tensor_copy(s2rep[:], reps2[:])

        # ==== phase 5: FFN (SwiGLU) ====
        with contextlib.ExitStack() as sc45:
            x2m = sc45.enter_context(tc.tile_pool(name="x2m", bufs=1))
            x2 = x2m.tile([P, KI_H, QR], dt.bfloat16)
            msb = x2m.tile([P, NF_FF, QR], dt.bfloat16)
            for nf in range(KI_H):
                nc.vector.tensor_tensor(x2[:, nf, :], h1sb[:, nf, :], s2rep[:], MUL)

            with tc.tile_pool(name="p5", bufs=1) as p5:
                for nf in range(NF_FF):
                    wtg = p5.tile([P, KI_H, P], dt.bfloat16, tag="wg", bufs=2)
                    nc.sync.dma_start(wtg[:], w_g[nf])
                    pg = ps.tile([P, QR], dt.float32, tag="mm")
                    for ki in range(KI_H):
                        nc.tensor.matmul(
                            pg[:], wtg[:, ki, :], x2[:, ki, :],
                            start=(ki == 0), stop=(ki == KI_H - 1),
                        )
                    gs = tmp.tile([P, QR], dt.bfloat16, tag="sq")
                    nc.scalar.activation(out=gs[:], in_=pg[:], func=AF.Silu)
                    wtu = p5.tile([P, KI_H, P], dt.bfloat16, tag="wu", bufs=2)
                    nc.sync.dma_start(wtu[:], w_u[nf])
                    pu = ps.tile([P, QR], dt.float32, tag="mm")
                    for ki in range(KI_H):
                        nc.tensor.matmul(
                            pu[:], wtu[:, ki, :], x2[:, ki, :],
                            start=(ki == 0), stop=(ki == KI_H - 1),
                        )
                    nc.vector.tensor_tensor(msb[:, nf, :], pu[:], gs[:], MUL)

                for nf in range(KI_H):
                    pt = ps.tile([P, QR], dt.float32, tag="mm")
                    for half in range(2):
                        wt = p5.tile([P, NF_FF // 2, P], dt.bfloat16, tag="wd", bufs=2)
                        nc.sync.dma_start(wt[:], w_d[nf, :, half * 32 : (half + 1) * 32, :])
                        for ki in range(NF_FF // 2):
                            kk = half * 32 + ki
                            nc.tensor.matmul(
                                pt[:], wt[:, ki, :], msb[:, kk, :],
                                start=(kk == 0), stop=(kk == NF_FF - 1),
                            )
                    ot = p5.tile([P, QR], dt.float32, tag="h1t", bufs=2)
                    nc.vector.tensor_tensor(ot[:], pt[:], h1sb[:, nf, :], ADD)
                    nc.sync.dma_start(out[nf * P : (nf + 1) * P, :], ot[:])

    return nc


# ---------------------------------------------------------------------------
# host-side packing
# ---------------------------------------------------------------------------
def _deint_perm():
    # deinterleave: out[i] = in[2i] (i<32), in[2(i-32)+1] (i>=32)
    return np.concatenate([np.arange(0, ROPE, 2), np.arange(1, ROPE, 2)])


def _pack_lhst(w, nki, nnf, nfree=P):
    # w [nki*P, nnf*nfree] -> [nnf, P, nki, nfree]
    return np.ascontiguousarray(
        w.reshape(nki, P, nnf, nfree).transpose(2, 1, 0, 3).astype(BF16)
    )


def _fp8(x):
    # TRN FP8_EXP4 matches OCP e4m3 bit patterns for |x| <= 240
    return np.clip(x, -240.0, 240.0).astype(ml_dtypes.float8_e4m3)


def _prep_shared(inputs):
    perm = _deint_perm()
    ln1 = inputs["ln1_w"].astype(np.float32)
    qaln = inputs["q_a_ln_w"].astype(np.float32)
    kvln = inputs["kv_a_ln_w"].astype(np.float32)
    ln2 = inputs["ln2_w"].astype(np.float32)

    w_qa = inputs["q_a_kernel"].astype(np.float32) * ln1[:, None]
    w_kva = inputs["kv_a_kernel"].astype(np.float32) * ln1[:, None]
    w_kva = w_kva.copy()
    # 16x on the rope cols: k_pe is stored fp8 pre-scaled for DoubleRow
    w_kva[:, KV_LORA:] = w_kva[:, KV_LORA:][:, perm] * 16.0
    w_qb = inputs["q_b_kernel"].astype(np.float32) * qaln[:, None]
    w_qb = w_qb.copy()
    for h in range(NH):
        blk = slice(h * QHD + NOPE, (h + 1) * QHD)
        w_qb[:, blk] = w_qb[:, blk][:, perm]
    w_kvb = inputs["kv_b_kernel"].astype(np.float32) * kvln[:, None]
    w_o = inputs["o_kernel"].astype(np.float32)
    w_g = inputs["gate_kernel"].astype(np.float32) * ln2[:, None]
    w_u = inputs["up_kernel"].astype(np.float32) * ln2[:, None]
    w_d = inputs["down_kernel"].astype(np.float32)

    # diagonal-block causal mask, identical on every core:
    # key (kt*128+p) visible to query q  <=>  kt*128+p <= q
    kp = np.arange(P)[:, None]
    qf = np.arange(QR)[None, :]
    mdg = np.zeros((P, TQ, QR), dtype=BF16)
    for kt in range(TQ):
        mdg[:, kt, :] = ((kt * P + kp) <= qf).astype(BF16)

    shared = {
        # DoubleRow fp8: K-tile pairs (2g+i) packed on the middle axes, 16x
        # pre-scaled for fp8 range
        "w_qa": np.ascontiguousarray(
            _fp8(w_qa.reshape(KI_H, P, KI_QL, P).transpose(2, 1, 0, 3) * 16.0)
            .reshape(KI_QL, P, KI_H // 2, 2, P)
        ),
        "w_qb": np.ascontiguousarray(
            _fp8(w_qb.reshape(KI_QL, P, NH, QHD).transpose(2, 1, 0, 3) * 16.0)
            .reshape(NH, P, KI_QL // 2, 2, QHD)
        ),
        # w_kva resident: [P, KI_H, 576]
        "w_kva": np.ascontiguousarray(
            w_kva.reshape(KI_H, P, KV_LORA + ROPE).transpose(1, 0, 2).astype(BF16)
        ),
        # w_kvb split into k/v halves, packed per head-group of 4:
        # [hg, p, lt, hh*128+c]
        # 16x: k_nope lands in fp8 pre-scaled for the DoubleRow score matmul;
        # latent K-tile pairs packed for the DoubleRow k_nope build
        "w_kv_k": np.ascontiguousarray(
            _fp8(
                (w_kvb.reshape(KI_KVL, P, NH // 4, 4, 2, 128)[:, :, :, :, 0, :] * 16.0)
                .transpose(2, 1, 0, 3, 4)
                .reshape(NH // 4, P, KI_KVL, 512)
            ).reshape(NH // 4, P, KI_KVL // 2, 2, 512)
        ),
        "w_kv_v": np.ascontiguousarray(
            w_kvb.reshape(KI_KVL, P, NH // 4, 4, 2, 128)[:, :, :, :, 1, :]
            .transpose(2, 1, 0, 3, 4)
            .reshape(NH // 4, P, KI_KVL, 512)
            .astype(BF16)
        ),
        # w_o: [KI_H(nf), P, NH, VHD]
        "w_o": np.ascontiguousarray(
            w_o.reshape(NH, VHD, KI_H, P).transpose(2, 1, 0, 3).astype(BF16)
        ),
        "w_g": _pack_lhst(w_g, KI_H, NF_FF),
        "w_u": _pack_lhst(w_u, KI_H, NF_FF),
        "w_d": _pack_lhst(w_d, NF_FF, KI_H),
        "maskdiag": mdg,
    }
    return shared


def _prep_batch(inputs, b):
    hid = np.asarray(inputs["hidden_states"][b], dtype=np.float32)  # [S, H]
    hT = np.ascontiguousarray(hid.T)  # [H, S]
    pos = np.asarray(inputs["position_ids"][b]).astype(np.int64)
    cos_g = np.asarray(inputs["cos"], dtype=np.float32)[pos][:, :32]  # [S, 32]
    sin_g = np.asarray(inputs["sin"], dtype=np.float32)[pos][:, :32]
    # [64] rows: cos duplicated; sin with rotate-half signs baked in
    cos2 = np.concatenate([cos_g, cos_g], axis=1)  # [S, 64]
    sin2 = np.concatenate([-sin_g, sin_g], axis=1)
    return hT, np.ascontiguousarray(cos2.T), np.ascontiguousarray(sin2.T)


def _core_colmask(j):
    # multiplicative mask per key tile: block b holds original chunk (j+b)%4.
    # past chunks (< j): 1 (fully visible); future (> j): 0 (masked).
    # block 0 (diagonal) uses the shared banded mask instead.
    cb = np.ones((P, TK), dtype=np.float32)
    for b in range(1, NBLK):
        c = (j + b) % NBLK
        if c > j:
            cb[:, b * TQ : (b + 1) * TQ] = 0.0
    return cb


def kernel(**inputs) -> np.ndarray:
    import concourse.bass as bass  # noqa: F401  (env check)
    from concourse.bass_utils import run_bass_kernel_spmd

    if "nc" not in _COMPILED:
        _COMPILED["nc"] = _build_nc()
    nc = _COMPILED["nc"]

    shared = _prep_shared(inputs)
    in_maps = []
    per_batch = [_prep_batch(inputs, b) for b in range(B)]
    hTb_cache = {}
    for c in range(8):
        b, j = c // 4, c % 4
        hT, cosT, sinT = per_batch[b]
        if b not in hTb_cache:
            hTb_cache[b] = hT.astype(BF16)
        hTbf = hTb_cache[b]
        q0 = j * QR
        rot = [((j + k) % NBLK) for k in range(NBLK)]  # block b -> orig chunk
        in_map = dict(shared)
        # key blocks 1..3 in rotated order (block 0 == the q slice, resident)
        in_map["hTb"] = np.ascontiguousarray(
            np.concatenate([hTbf[:, c_ * QR : (c_ + 1) * QR] for c_ in rot[1:]], axis=1)
        )
        in_map["hTqb"] = np.ascontiguousarray(hTbf[:, q0 : q0 + QR])
        in_map["hTq"] = np.ascontiguousarray(hT[:, q0 : q0 + QR])
        in_map["cosT"] = np.ascontiguousarray(
            np.concatenate([cosT[:, c_ * QR : (c_ + 1) * QR] for c_ in rot], axis=1)
        ).astype(BF16)
        in_map["sinT"] = np.ascontiguousarray(
            np.concatenate([sinT[:, c_ * QR : (c_ + 1) * QR] for c_ in rot], axis=1)
        ).astype(BF16)
        in_map["cosTq"] = np.ascontiguousarray(cosT[:, q0 : q0 + QR])
        in_map["sinTq"] = np.ascontiguousarray(sinT[:, q0 : q0 + QR])
        in_map["colmask"] = _core_colmask(j)
        in_maps.append(in_map)

    res = run_bass_kernel_spmd(nc, in_maps, core_ids=list(range(8)))
    globals()["LAST_RESULT"] = res

    out = np.empty((B, S, H), dtype=np.float32)
    for c in range(8):
        b, j = c // 4, c % 4
        out[b, j * QR : (j + 1) * QR, :] = res.results[c]["out"].T
    return out


# revision 71
# speedup vs baseline: 2.0193x; 2.0193x over previous
"""DeepseekV2 decoder layer (MLA attention + SwiGLU MLP) on 8 TRN2 NeuronCores.

Sharding: core c -> batch b = c//4, query rows [j*512, (j+1)*512) with j = c%4.
Every core computes the full-sequence KV latents for its batch (cheap shared
latents, exactly MLA's design), its own 512 query rows through attention +
o_proj + FFN, and returns its 512 output rows. No collectives.

All cores run one identical SPMD program; per-core position enters only
through input data (causal masks, sliced hidden/rope tables).

On-device layout is feature-major (activations transposed, features on
partitions): for y = x @ W the device computes y^T = matmul(lhsT=W_tile,
rhs=x^T_tile) accumulating K-tiles in PSUM. RMSNorm weights are folded into
adjacent weight matrices on the host; RMSNorm 1/rms column scales are applied
when copying matmul outputs from PSUM to SBUF; cross-partition reductions use
ones-vector matmuls.

Keys are processed in per-core ROTATED chunk order: key block b holds the
original 512-column chunk (j+b)%4, so block 0 is the core's own query-aligned
chunk.  Block 0's hidden slice and ln1 stats are already resident from the q
path (no DMA, no stats), the causal mask becomes a shared diagonal band for
block 0 plus a constant 0/-50 exp-bias per key tile for the other blocks, and
the kv phase starts with zero load latency.  Stat/normalize chains are
software-pipelined so the in-order PE never waits on scalar/vector results;
softmax denominators pre-reduce groups of 4 exp tiles on the vector engine.
"""

import json

import numpy as np
import ml_dtypes

B, S, H = 2, 2048, 2048
NH = 16
Q_LORA = 1536
KV_LORA = 512
NOPE = 128
ROPE = 64
QHD = NOPE + ROPE  # 192
VHD = 128
FF = 8192
EPS = 1e-6
P = 128
QR = 512  # query rows per core
TK = S // P  # 16 key tiles
TQ = QR // P  # 4
KI_H = H // P  # 16
KI_QL = Q_LORA // P  # 12
KI_KVL = KV_LORA // P  # 4
NF_FF = FF // P  # 64
NBLK = S // QR  # 4 key blocks
ATTN_SCALE = QHD ** -0.5
MASK_NEG = -50.0

BF16 = ml_dtypes.bfloat16

_COMPILED = {}


# ---------------------------------------------------------------------------
# compiler workaround: this container's walrus rejects >1 sem wait per
# instruction; split extra waits onto single-wait NoOps.
# ---------------------------------------------------------------------------
def _install_multiwait_fix(bass):
    if getattr(bass.Bass, "_multiwait_fix_installed", False):
        return
    orig = bass.Bass.to_json_bytes

    def _split(m):
        for f in m.get("functions", []):
            for b in f.get("blocks", []):
                out = []
                for inst in b.get("instructions", []):
                    si = inst.get("sync_info") or {}
                    waits = si.get("on_wait") or []
                    if len(waits) > 1:
                        for k, w in enumerate(waits[:-1]):
                            out.append(
                                {
                                    "debug": inst.get("debug", 0),
                                    "engine": inst["engine"],
                                    "ins": [],
                                    "name": f"{inst['name']}_w{k}",
                                    "opcode": "NoOp",
                                    "outs": [],
                                    "sync_info": {"on_update": [], "on_wait": [w]},
                                }
                            )
                        si["on_wait"] = [waits[-1]]
                    out.append(inst)
                b["instructions"] = out
        return m

    def patched(self):
        raw = orig(self)
        try:
            return json.dumps(_split(json.loads(raw))).encode()
        except Exception:
            return raw

    bass.Bass.to_json_bytes = patched
    bass.Bass._multiwait_fix_installed = True


def _install_drain_fix(tile, ScopedClock, VectorClock):
    if getattr(tile.TileContext, "_drain_fix_installed", False):
        return

    def _drain_and_barrier(self, tick_clock, wait_clock):
        gc = tick_clock.global_clock
        n = len(gc)
        for p in range(n):
            t = gc[p]
            if t > 0:
                vc = VectorClock([0] * n)
                vc.require_at_least(p, t)
                d = self.nc.sync.drain()
                wait_clock.add_sem_waits(d.ins, ScopedClock({None: vc}))
        self.nc.all_engine_barrier()
        popped = self.nc._tile_sem_poison_stack.pop()
        assert popped is self._sem_poison
        self.nc.clear_and_free_semaphores(list(self.sems.allocated().values()))
        self.nc.all_engine_barrier()

    tile.TileContext._drain_and_barrier = _drain_and_barrier
    tile.TileContext._drain_fix_installed = True


# ---------------------------------------------------------------------------
# device program
# ---------------------------------------------------------------------------
def _build_nc():
    import concourse.bass as bass
    import concourse.mybir as mybir
    import concourse.tile as tile
    from concourse.vector_clock import ScopedClock, VectorClock

    _install_multiwait_fix(bass)
    _install_drain_fix(tile, ScopedClock, VectorClock)

    dt = mybir.dt
    AF = mybir.ActivationFunctionType
    MUL = mybir.AluOpType.mult
    ADD = mybir.AluOpType.add
    SUB = mybir.AluOpType.subtract

    nc = bass.Bass()

    # register EPS so float bias=EPS works on the scalar engine
    _eps_t = nc.alloc_sbuf_tensor(f"const-float32-{EPS}", [128, 1], dt.float32)
    nc.gpsimd.memset(_eps_t.ap(), EPS)
    nc.const_aps.aps[(dt.float32, EPS)] = _eps_t.ap()
    nc.all_engine_barrier()

    # ---- inputs ----
    # hTb holds key blocks 1..3 only (block 0 == the resident q slice)
    hTb = nc.dram_tensor("hTb", [H, S - QR], dt.bfloat16, kind="ExternalInput")
    hTqb = nc.dram_tensor("hTqb", [H, QR], dt.bfloat16, kind="ExternalInput")
    hTq = nc.dram_tensor("hTq", [H, QR], dt.float32, kind="ExternalInput")
    # rope tables are [64] rows: cos duplicated, sin with the rotate-half
    # signs baked in ([-sin; sin]) so rope runs as 3 full-width DVE ops
    cosT = nc.dram_tensor("cosT", [ROPE, S], dt.bfloat16, kind="ExternalInput")
    sinT = nc.dram_tensor("sinT", [ROPE, S], dt.bfloat16, kind="ExternalInput")
    cosTq = nc.dram_tensor("cosTq", [ROPE, QR], dt.float32, kind="ExternalInput")
    sinTq = nc.dram_tensor("sinTq", [ROPE, QR], dt.float32, kind="ExternalInput")
    maskdiag = nc.dram_tensor("maskdiag", [P, TQ, QR], dt.bfloat16, kind="ExternalInput")
    colmask = nc.dram_tensor("colmask", [P, TK], dt.float32, kind="ExternalInput")
    w_qa = nc.dram_tensor("w_qa", [KI_QL, P, KI_H // 2, 2, P], dt.float8e4, kind="ExternalInput")
    w_qb = nc.dram_tensor("w_qb", [NH, P, KI_QL // 2, 2, QHD], dt.float8e4, kind="ExternalInput")
    w_kva = nc.dram_tensor("w_kva", [P, KI_H, KV_LORA + ROPE], dt.bfloat16, kind="ExternalInput")
    w_kv_k = nc.dram_tensor("w_kv_k", [NH // 4, P, KI_KVL // 2, 2, 512], dt.float8e4, kind="ExternalInput")
    w_kv_v = nc.dram_tensor("w_kv_v", [NH // 4, P, KI_KVL, 512], dt.bfloat16, kind="ExternalInput")
    w_o = nc.dram_tensor("w_o", [KI_H, P, NH, VHD], dt.bfloat16, kind="ExternalInput")
    w_g = nc.dram_tensor("w_g", [NF_FF, P, KI_H, P], dt.bfloat16, kind="ExternalInput")
    w_u = nc.dram_tensor("w_u", [NF_FF, P, KI_H, P], dt.bfloat16, kind="ExternalInput")
    w_d = nc.dram_tensor("w_d", [KI_H, P, NF_FF, P], dt.bfloat16, kind="ExternalInput")
    out = nc.dram_tensor("out", [H, QR], dt.float32, kind="ExternalOutput")

    def rsqrt_stat(tmp, acc, denom):
        # 1/sqrt(mean + eps) from a [1, N] PSUM sum-of-squares accumulator
        s = tmp.tile([1, acc.shape[-1]], dt.float32, tag="stat", bufs=2)
        nc.scalar.activation(out=s[:], in_=acc[:], func=AF.Sqrt, bias=EPS, scale=1.0 / denom)
        nc.vector.reciprocal(s[:], s[:])
        return s

    import contextlib

    with tile.TileContext(nc) as tc, contextlib.ExitStack() as top:
        tp = lambda **kw: top.enter_context(tc.tile_pool(**kw))
        ones = tp(name="ones", bufs=1)
        tmp = tp(name="tmp", bufs=3)
        ld = tp(name="ld", bufs=3)
        ps = tp(name="ps", bufs=3, space="PSUM")
        ps_acc = tp(name="ps_acc", bufs=1, space="PSUM")
        # prefetch tiles; their loads are issued just after the xqbf loads
        # (first-needed data first in the DMA queues)
        pre = tp(name="pre", bufs=1)
        wkh0 = pre.tile([P, KI_KVL // 2, 2, 512], dt.float8e4)
        wvh0 = pre.tile([P, KI_KVL, 512], dt.bfloat16)
        wkva = pre.tile([P, KI_H, KV_LORA + ROPE], dt.bfloat16)

        def issue_prefetch():
            nc.sync.dma_start(wkva[:], w_kva[:])
            nc.sync.dma_start(wkh0[:], w_kv_k[0])
            nc.sync.dma_start(wvh0[:], w_kv_v[0])
        # h1 (residual after attention) stays resident in SBUF
        h1_pool = tp(name="h1_pool", bufs=1)
        h1sb = h1_pool.tile([P, KI_H, QR], dt.bfloat16)
        s2rep = h1_pool.tile([P, QR], dt.float32)

        # [1, P] row for partition replication (K=1 matmul),
        # [P, 1] column for cross-partition reduction (M=1 matmul).
        ones_f32 = ones.tile([1, P], dt.float32)
        nc.vector.memset(ones_f32[:], 1.0)
        ones_bf = ones.tile([P, 1], dt.bfloat16)
        nc.vector.memset(ones_bf[:], 1.0)

        with contextlib.ExitStack() as mid:
            lat = mid.enter_context(tc.tile_pool(name="lat", bufs=1))
            ckv = lat.tile([P, KI_KVL, S], dt.bfloat16)  # normalized kv latents
            # fp8 copy of the latents packed for DoubleRow k_nope builds
            # (v projections keep the bf16 ckv for accuracy)
            ckv8 = lat.tile([P, 2, KI_KVL // 2, S], dt.float8e4)
            kpe = lat.tile([ROPE, S], dt.float8e4)  # roped shared 16*key-pe
            mdg = lat.tile([P, TQ, QR], dt.bfloat16)  # block-0 diagonal mask
            cmask = lat.tile([P, TK], dt.float32)  # 1/0 mask per key tile
            attn_pool = mid.enter_context(tc.tile_pool(name="attn_pool", bufs=1))
            attn = attn_pool.tile([P, NH, QR], dt.bfloat16)
            qnp = mid.enter_context(tc.tile_pool(name="qnp", bufs=1))
            # q packed for DoubleRow fp8 scores: slot 0 = 16*q_nope,
            # slot 1 = 16*q_pe on partitions 0-63, zero pad on 64-127
            qpk = qnp.tile([P, 2, NH, QR], dt.float8e4)
            nc.vector.memset(qpk[ROPE:, 1, :, :], 0.0)

            with contextlib.ExitStack() as scA:
                pA = scA.enter_context(tc.tile_pool(name="pA", bufs=1))
                xqbf = pA.tile([P, KI_H, QR], dt.bfloat16)
                s1qrep = pA.tile([P, QR], dt.float32)

                # ==== phase 1: q path (kv-block loads stream in behind) ====
                accq = ps_acc.tile([1, QR], dt.float32, tag="acc")
                for ki in range(KI_H):
                    nc.sync.dma_start(xqbf[:, ki, :], hTqb[ki * P : (ki + 1) * P, :])
                    sq = tmp.tile([P, QR], dt.bfloat16, tag="sq")
                    nc.vector.tensor_tensor(sq[:], xqbf[:, ki, :], xqbf[:, ki, :], MUL)
                    nc.tensor.matmul(
                        accq[:], ones_bf[:], sq[:], start=(ki == 0), stop=(ki == KI_H - 1)
                    )
                s1q = rsqrt_stat(tmp, accq, H)
                issue_prefetch()
                nc.sync.dma_start(mdg[:], maskdiag[:])
                nc.sync.dma_start(cmask[:], colmask[:])

                with tc.tile_pool(name="p2", bufs=1) as p2:
                    # 16*x packed fp8 for DoubleRow q_a (ln1 q stats and the
                    # kv block-0 projections keep using the bf16 xqbf)
                    xq8 = p2.tile([P, 2, KI_H // 2, QR], dt.float8e4)
                    for ki in range(KI_H):
                        nc.vector.tensor_scalar_mul(
                            xq8[:, ki % 2, ki // 2, :], xqbf[:, ki, :], 16.0
                        )
                    # qlat8 = 16*qlat: psum is 256*(x@w_qa); (1/16)/rms scale
                    qlat = p2.tile([P, 2, KI_QL // 2, QR], dt.float8e4)
                    qacc = ps_acc.tile([1, QR], dt.float32, tag="acc")
                    prev_sq = None
                    for nf in range(KI_QL):
                        wt = p2.tile([P, KI_H // 2, 2, P], dt.float8e4, tag="wqa", bufs=2)
                        nc.sync.dma_start(wt[:], w_qa[nf])
                        pt = ps.tile([P, QR], dt.float32, tag="mm")
                        for g in range(KI_H // 2):
                            nc.tensor.matmul(
                                pt[:],
                                wt[:, g, :, :],
                                xq8[:, :, g, :],
                                start=(g == 0),
                                stop=(g == KI_H // 2 - 1),
                                perf_mode=mybir.MatmulPerfMode.DoubleRow,
                            )
                        if nf == 0:
                            # replicate 1/rms after the first q_a chain so the
                            # PE never waits on the sqrt/reciprocal latency
                            repq = ps.tile([P, QR], dt.float32, tag="mm")
                            nc.tensor.matmul(repq[:], ones_f32[:], s1q[:], start=True, stop=True)
                            nc.vector.tensor_copy(s1qrep[:], repq[:])
                        if prev_sq is not None:
                            nc.tensor.matmul(
                                qacc[:], ones_bf[:], prev_sq[:],
                                start=(nf == 1), stop=False,
                            )
                        nc.vector.scalar_tensor_tensor(
                            qlat[:, nf % 2, nf // 2, :], pt[:], 1.0 / 16.0,
                            s1qrep[:], MUL, MUL,
                        )
                        sq = tmp.tile([P, QR], dt.bfloat16, tag="sq")
                        nc.vector.tensor_tensor(
                            sq[:], qlat[:, nf % 2, nf // 2, :],
                            qlat[:, nf % 2, nf // 2, :], MUL,
                        )
                        prev_sq = sq

                    sqrep = p2.tile([P, QR], dt.float32)
                    cosq = p2.tile([ROPE, QR], dt.float32)
                    sinq = p2.tile([ROPE, QR], dt.float32)
                    nc.sync.dma_start(cosq[:], cosTq[:])
                    nc.sync.dma_start(sinq[:], sinTq[:])

                    def qb_mm(h):
                        wt = p2.tile([P, KI_QL // 2, 2, QHD], dt.float8e4, tag="wqb", bufs=2)
                        nc.sync.dma_start(wt[:], w_qb[h])
                        pt = ps.tile([P, QR], dt.float32, tag="mm")
                        for g in range(KI_QL // 2):
                            nc.tensor.matmul(
                                pt[:], wt[:, g, :, :NOPE], qlat[:, :, g, :],
                                start=(g == 0), stop=(g == KI_QL // 2 - 1),
                                perf_mode=mybir.MatmulPerfMode.DoubleRow,
                            )
                        if h == 0:
                            # finish the q_a_ln stat chain + replicate between
                            # head 0's two matmul chains (latency hiding).
                            # qacc holds 256*sum(qlat^2), so 1/sqrt(mean*256)
                            # = (1/16)/rms; psum q is 256x -> qpk = 16*q/rms
                            nc.tensor.matmul(
                                qacc[:], ones_bf[:], prev_sq[:], start=False, stop=True
                            )
                            sqv = rsqrt_stat(tmp, qacc, Q_LORA)
                        pt2 = ps.tile([ROPE, QR], dt.float32, tag="mm")
                        for g in range(KI_QL // 2):
                            nc.tensor.matmul(
                                pt2[:], wt[:, g, :, NOPE:QHD], qlat[:, :, g, :],
                                start=(g == 0), stop=(g == KI_QL // 2 - 1),
                                perf_mode=mybir.MatmulPerfMode.DoubleRow,
                            )
                        if h == 0:
                            repsq = ps.tile([P, QR], dt.float32, tag="mm")
                            nc.tensor.matmul(repsq[:], ones_f32[:], sqv[:], start=True, stop=True)
                            nc.vector.tensor_copy(sqrep[:], repsq[:])
                            # fold q_a_ln scale into the q rope tables
                            nc.vector.tensor_tensor(cosq[:], cosq[:], sqrep[:ROPE, :], MUL)
                            nc.vector.tensor_tensor(sinq[:], sinq[:], sqrep[:ROPE, :], MUL)
                        return pt, pt2

                    def qb_dve(h, pt, pt2):
                        nc.vector.tensor_tensor(qpk[:, 0, h, :], pt[:], sqrep[:], MUL)
                        pesq = p2.tile([ROPE, QR], dt.float32, tag="pes", bufs=2)
                        nc.vector.tensor_copy(pesq[:], pt2[:])
                        # swapped halves for rotate-half (signs live in sinq)
                        xsw = p2.tile([ROPE, QR], dt.float32, tag="x2h", bufs=2)
                        nc.sync.dma_start(xsw[:32, :], pesq[32:, :])
                        nc.sync.dma_start(xsw[32:, :], pesq[:32, :])
                        t1 = p2.tile([ROPE, QR], dt.float32, tag="t1", bufs=2)
                        t2 = p2.tile([ROPE, QR], dt.float32, tag="t2", bufs=2)
                        nc.vector.tensor_tensor(t1[:], pesq[:], cosq[:], MUL)
                        nc.vector.tensor_tensor(t2[:], xsw[:], sinq[:], MUL)
                        nc.vector.tensor_tensor(qpk[:ROPE, 1, h, :], t1[:], t2[:], ADD)

                    for h in range(NH):
                        pt, pt2 = qb_mm(h)
                        qb_dve(h, pt, pt2)

                # ==== phase 2: kv latents per 512-col key block ============
                # block 0 reuses xqbf + s1qrep (no loads, no ln1 stats); the
                # stat/normalize chains are software-pipelined across blocks.
                with tc.tile_pool(name="pB", bufs=1) as pB:

                    def wkva_sl(ki, csl):
                        return wkva[:, ki, csl]
                    cosb = pB.tile([ROPE, S], dt.bfloat16)
                    sinb = pB.tile([ROPE, S], dt.bfloat16)
                    nc.sync.dma_start(cosb[:], cosT[:])
                    nc.sync.dma_start(sinb[:], sinT[:])

                    xcs = {}
                    s1rs = {}
                    kvaccs = {}
                    pend_rep1 = {}
                    pend_repkv = {}

                    def kv_stats(t):
                        # ln1 sum-of-squares for loaded block t (PE chain)
                        xc = pB.tile([P, KI_H, 512], dt.bfloat16, tag="xc", bufs=2)
                        xcs[t] = xc
                        acc = ps_acc.tile([1, 512], dt.float32, tag="acc")
                        for ki in range(KI_H):
                            nc.sync.dma_start(
                                xc[:, ki, :], hTb[ki * P : (ki + 1) * P, (t - 1) * 512 : t * 512]
                            )
                            sq = tmp.tile([P, 512], dt.bfloat16, tag="sq")
                            nc.vector.tensor_tensor(sq[:], xc[:, ki, :], xc[:, ki, :], MUL)
                            nc.tensor.matmul(
                                acc[:], ones_bf[:], sq[:], start=(ki == 0), stop=(ki == KI_H - 1)
                            )
                        pend_rep1[t] = rsqrt_stat(tmp, acc, H)

                    def kv_rep1(t):
                        s1 = pend_rep1.pop(t)
                        rep1 = ps.tile([P, 512], dt.float32, tag="mm")
                        nc.tensor.matmul(rep1[:], ones_f32[:], s1[:], start=True, stop=True)
                        s1r = tmp.tile([P, 512], dt.float32, tag="s1r", bufs=2)
                        nc.vector.tensor_copy(s1r[:], rep1[:])
                        s1rs[t] = s1r

                    def kv_proj_mm(t, nf):
                        # one latent K-tile projection for block t (PE only)
                        rhs = xqbf if t == 0 else xcs[t]
                        pt = ps.tile([P, 512], dt.float32, tag="mm")
                        for ki in range(KI_H):
                            nc.tensor.matmul(
                                pt[:],
                                wkva_sl(ki, slice(nf * P, (nf + 1) * P)),
                                rhs[:, ki, :],
                                start=(ki == 0),
                                stop=(ki == KI_H - 1),
                            )
                        return pt

                    def kv_proj_dve(t, nf, pt):
                        # ln1 1/rms column scale on the way out of PSUM
                        tsl = slice(t * 512, (t + 1) * 512)
                        s1r = s1qrep if t == 0 else s1rs[t]
                        nc.vector.tensor_tensor(ckv[:, nf, tsl], pt[:], s1r[:], MUL)
                        sq = tmp.tile([P, 512], dt.bfloat16, tag="sq")
                        nc.vector.tensor_tensor(sq[:], ckv[:, nf, tsl], ckv[:, nf, tsl], MUL)
                        return sq

                    def kv_acc_mm(t, nf, sq):
                        if t not in kvaccs:
                            kvaccs[t] = ps_acc.tile(
                                [1, 512], dt.float32, tag="acc", name="kvacc"
                            )
                        nc.tensor.matmul(
                            kvaccs[t][:], ones_bf[:], sq[:],
                            start=(nf == 0), stop=(nf == KI_KVL - 1),
                        )
                        if nf == KI_KVL - 1:
                            pend_repkv[t] = rsqrt_stat(tmp, kvaccs.pop(t), KV_LORA)

                    def kv_rope(t):
                        tsl = slice(t * 512, (t + 1) * 512)
                        rhs = xqbf if t == 0 else xcs[t]
                        s1r = s1qrep if t == 0 else s1rs[t]
                        pt = ps.tile([ROPE, 512], dt.float32, tag="mm")
                        for ki in range(KI_H):
                            nc.tensor.matmul(
                                pt[:],
                                wkva_sl(ki, slice(KV_LORA, KV_LORA + ROPE)),
                                rhs[:, ki, :],
                                start=(ki == 0),
                                stop=(ki == KI_H - 1),
                            )
                        pes = pB.tile([ROPE, 512], dt.float32, tag="pes", bufs=1)
                        nc.vector.tensor_tensor(pes[:], pt[:], s1r[:ROPE, :], MUL)
                        # swapped halves for rotate-half (signs live in sinb)
                        xsw = pB.tile([ROPE, 512], dt.float32, tag="x2h", bufs=1)
                        nc.sync.dma_start(xsw[:32, :], pes[32:, :])
                        nc.sync.dma_start(xsw[32:, :], pes[:32, :])
                        t1 = pB.tile([ROPE, 512], dt.float32, tag="t1", bufs=1)
                        t2 = pB.tile([ROPE, 512], dt.float32, tag="t2", bufs=1)
                        nc.vector.tensor_tensor(t1[:], pes[:], cosb[:, tsl], MUL)
                        nc.vector.tensor_tensor(t2[:], xsw[:], sinb[:, tsl], MUL)
                        nc.vector.tensor_tensor(kpe[:, tsl], t1[:], t2[:], ADD)

                    def kv_scale(t):
                        # apply the kv_a rmsnorm 1/rms to block t's latents
                        skv = pend_repkv.pop(t)
                        repkv = ps.tile([P, 512], dt.float32, tag="mm")
                        nc.tensor.matmul(repkv[:], ones_f32[:], skv[:], start=True, stop=True)
                        rkv = tmp.tile([P, 512], dt.float32, tag="s1r", bufs=2)
                        nc.vector.tensor_copy(rkv[:], repkv[:])
                        tsl = slice(t * 512, (t + 1) * 512)
                        for nf in range(KI_KVL):
                            nc.vector.tensor_tensor(ckv[:, nf, tsl], ckv[:, nf, tsl], rkv[:], MUL)
                            nc.vector.tensor_copy(ckv8[:, nf % 2, nf // 2, tsl], ckv[:, nf, tsl])

                    def kv_block_work(t):
                        # projections with the kvacc chain trailing one K-tile;
                        # for loaded blocks the 1/rms replicate slots in after
                        # the first projection chain (hides sqrt+recip latency)
                        sqs = []
                        for nf in range(KI_KVL):
                            pt = kv_proj_mm(t, nf)
                            if nf == 0 and t > 0:
                                kv_rep1(t)
                            sqs.append(kv_proj_dve(t, nf, pt))
                            if nf >= 1:
                                kv_acc_mm(t, nf - 1, sqs[nf - 1])
                        # kvacc (and its sqrt+reciprocal) ahead of the rope
                        # DVE chain, so the reciprocal isn't queued behind it
                        kv_acc_mm(t, KI_KVL - 1, sqs[-1])
                        kv_rope(t)

                    kv_block_work(0)
                    for t in range(1, NBLK):
                        kv_stats(t)
                        kv_scale(t - 1)
                        kv_block_work(t)
                    # kv_scale(3) is deferred into the attention phase

            # ==== phase 3: attention ====
            with tc.tile_pool(name="p3", bufs=1) as p3, tc.tile_pool(
                name="ps_att", bufs=1, space="PSUM"
            ) as ps_att:
                # previous head's softmax normalize: se is staged to SBUF on
                # the scalar engine, replicated by the PE (no slow input),
                # and the reciprocal runs FULL-WIDTH on the replicated
                # [128,512] tile (~270ns) instead of on [1,512] (~3.3us).
                pending = []  # (h, av, se_sb)

                def flush_norm():
                    while pending:
                        h_, av_, ses = pending.pop(0)
                        repr_ = ps.tile([P, QR], dt.float32, tag="mm")
                        nc.tensor.matmul(repr_[:], ones_f32[:], ses[:], start=True, stop=True)
                        rsb = tmp.tile([P, QR], dt.float32, tag="s1r", bufs=2)
                        nc.vector.reciprocal(rsb[:], repr_[:])
                        nc.vector.tensor_tensor(attn[:, h_, :], av_[:], rsb[:], MUL)

                for hg in range(NH // 4):
                    if hg == 0:
                        wkh, wvh = wkh0, wvh0
                    else:
                        wkh = p3.tile([P, KI_KVL // 2, 2, 512], dt.float8e4, tag="wkh", bufs=2)
                        nc.sync.dma_start(wkh[:], w_kv_k[hg])
                        wvh = p3.tile([P, KI_KVL, 512], dt.bfloat16, tag="wvh", bufs=2)
                        nc.sync.dma_start(wvh[:], w_kv_v[hg])
                    # v for 4 heads at once: v_rm[kpos, 4*VHD]
                    vsb = p3.tile([P, TK, 4 * VHD], dt.bfloat16, tag="vsb", bufs=2)
                    for kt in range(TK):
                        if hg == 0 and kt == 12:
                            # blocks 0-2 are done; finish block 3's kv norm
                            # while the PE is busy with the first 12 tiles
                            kv_scale(3)
                        pt = ps.tile([P, 4 * VHD], dt.float32, tag="mm")
                        for lt in range(KI_KVL):
                            nc.tensor.matmul(
                                pt[:],
                                ckv[:, lt, kt * P : (kt + 1) * P],
                                wvh[:, lt, :],
                                start=(lt == 0),
                                stop=(lt == KI_KVL - 1),
                            )
                        nc.vector.tensor_copy(vsb[:, kt, :], pt[:])
                    for hh in range(4):
                        h = hg * 4 + hh
                        # k packed for DoubleRow fp8: slot 0 = 16*k_nope
                        # (host-folded into w_kv_k), slot 1 = 16*k_pe + pad
                        ksb = p3.tile([P, 2, S], dt.float8e4, tag="ksb", bufs=2)
                        nc.vector.memset(ksb[ROPE:, 1, :], 0.0)
                        for t in range(NBLK):
                            pt = ps.tile([P, 512], dt.float32, tag="mm")
                            for g in range(KI_KVL // 2):
                                nc.tensor.matmul(
                                    pt[:],
                                    wkh[:, g, :, hh * P : (hh + 1) * P],
                                    ckv8[:, :, g, t * 512 : (t + 1) * 512],
                                    start=(g == 0),
                                    stop=(g == KI_KVL // 2 - 1),
                                    perf_mode=mybir.MatmulPerfMode.DoubleRow,
                                )
                            tsl = slice(t * 512, (t + 1) * 512)
                            nc.vector.tensor_copy(ksb[:, 0, tsl], pt[:])
                            nc.vector.tensor_copy(ksb[:ROPE, 1, tsl], kpe[:, tsl])
                        # scores / masked exp / attnV over all key tiles
                        av = ps_att.tile([P, QR], dt.float32, tag="av", bufs=2)
                        se = ps_att.tile([1, QR], dt.float32, tag="se", bufs=2)
                        # 2-deep software pipeline: emit av for kt-2 so the
                        # PE never stalls on the exp+mask chain.  The softmax
                        # denominator pre-reduces groups of GS exp tiles on
                        # the vector engine (4x fewer M=1 PE matmuls).
                        DELAY = 2
                        GS = 4
                        prs = {}

                        def _drain_kt(kt):
                            pr4, slot = prs.pop(kt)
                            nc.tensor.matmul(
                                av[:], vsb[:, kt, hh * VHD : (hh + 1) * VHD],
                                pr4[:, slot, :],
                                start=(kt == 0), stop=(kt == TK - 1),
                            )

                        pr4 = None
                        for kt in range(TK):
                            if kt == 4:
                                flush_norm()
                            g, slot = divmod(kt, GS)
                            if slot == 0:
                                pr4 = p3.tile([P, GS, QR], dt.bfloat16, tag="pr4", bufs=2)
                            sc = ps.tile([P, QR], dt.float32, tag="mm")
                            nc.tensor.matmul(
                                sc[:], ksb[:, :, kt * P : (kt + 1) * P],
                                qpk[:, :, h, :],
                                start=True, stop=True,
                                perf_mode=mybir.MatmulPerfMode.DoubleRow,
                            )
                            # scores carry 16*16 = 256x from the fp8 scaling
                            nc.scalar.activation(
                                out=pr4[:, slot, :], in_=sc[:], func=AF.Exp,
                                scale=ATTN_SCALE / 256.0,
                            )
                            if kt < TQ:
                                # block 0 = the diagonal block: banded mask
                                nc.vector.tensor_tensor(
                                    pr4[:, slot, :], pr4[:, slot, :], mdg[:, kt, :], MUL
                                )
                            else:
                                # other blocks all-past (1.0) or all-future
                                # (0.0); per-key-tile scalar on idle GPSIMD
                                nc.vector.tensor_scalar_mul(
                                    pr4[:, slot, :], pr4[:, slot, :],
                                    cmask[:, kt : kt + 1],
                                )
                            prs[kt] = (pr4, slot)
                            if kt >= DELAY:
                                _drain_kt(kt - DELAY)
                            if slot == GS - 1:
                                t01 = p3.tile([P, QR], dt.bfloat16, tag="t01", bufs=2)
                                t23 = p3.tile([P, QR], dt.bfloat16, tag="t23", bufs=2)
                                gsum = p3.tile([P, QR], dt.bfloat16, tag="gsum", bufs=2)
                                nc.vector.tensor_tensor(t01[:], pr4[:, 0, :], pr4[:, 1, :], ADD)
                                nc.vector.tensor_tensor(t23[:], pr4[:, 2, :], pr4[:, 3, :], ADD)
                                nc.vector.tensor_tensor(gsum[:], t01[:], t23[:], ADD)
                                nc.tensor.matmul(
                                    se[:], ones_bf[:], gsum[:],
                                    start=(g == 0), stop=(g == TK // GS - 1),
                                )
                        for kt in range(TK - DELAY, TK):
                            _drain_kt(kt)
                        se_sb = tmp.tile([1, QR], dt.float32, tag="stat", bufs=2)
                        nc.scalar.activation(out=se_sb[:], in_=se[:], func=AF.Copy)
                        pending.append((h, av, se_sb))
                flush_norm()

            # ==== phase 4: o_proj + residual + ln2 (h1 resident in SBUF) ====
            with tc.tile_pool(name="p4", bufs=1) as p4:
                oacc = ps_acc.tile([1, QR], dt.float32, tag="acc")
                for nf in range(KI_H):
                    wt = p4.tile([P, NH, VHD], dt.bfloat16, tag="wo", bufs=2)
                    nc.sync.dma_start(wt[:], w_o[nf])
                    pt = ps.tile([P, QR], dt.float32, tag="mm")
                    for kh in range(NH):
                        nc.tensor.matmul(
                            pt[:],
                            wt[:, kh, :],
                            attn[:, kh, :],
                            start=(kh == 0),
                            stop=(kh == NH - 1),
                        )
                    ht = ld.tile([P, QR], dt.float32, tag="hload")
                    nc.sync.dma_start(ht[:], hTq[nf * P : (nf + 1) * P, :])
                    nc.vector.tensor_tensor(h1sb[:, nf, :], pt[:], ht[:], ADD)
                    sq = tmp.tile([P, QR], dt.bfloat16, tag="sq")
                    nc.vector.tensor_tensor(sq[:], h1sb[:, nf, :], h1sb[:, nf, :], MUL)
                    nc.tensor.matmul(
                        oacc[:], ones_bf[:], sq[:], start=(nf == 0), stop=(nf == KI_H - 1)
                    )
                s2 = rsqrt_stat(tmp, oacc, H)
                reps2 = ps.tile([P, QR], dt.float32, tag="mm")
                nc.tensor.matmul(reps2[:], ones_f32[:], s2[:], start=True, stop=True)
                nc.vector.tensor_copy(s2rep[:], reps2[:])

        # ==== phase 5: FFN (SwiGLU) ====
        with contextlib.ExitStack() as sc45:
            x2m = sc45.enter_context(tc.tile_pool(name="x2m", bufs=1))
            x2 = x2m.tile([P, KI_H, QR], dt.bfloat16)
            msb = x2m.tile([P, NF_FF, QR], dt.bfloat16)
            for nf in range(KI_H):
                nc.vector.tensor_tensor(x2[:, nf, :], h1sb[:, nf, :], s2rep[:], MUL)

            with tc.tile_pool(name="p5", bufs=1) as p5:
                for nf in range(NF_FF):
                    wtg = p5.tile([P, KI_H, P], dt.bfloat16, tag="wg", bufs=2)
                    nc.sync.dma_start(wtg[:], w_g[nf])
                    pg = ps.tile([P, QR], dt.float32, tag="mm")
                    for ki in range(KI_H):
                        nc.tensor.matmul(
                            pg[:], wtg[:, ki, :], x2[:, ki, :],
                            start=(ki == 0), stop=(ki == KI_H - 1),
                        )
                    gs = tmp.tile([P, QR], dt.bfloat16, tag="sq")
                    nc.scalar.activation(out=gs[:], in_=pg[:], func=AF.Silu)
                    wtu = p5.tile([P, KI_H, P], dt.bfloat16, tag="wu", bufs=2)
                    nc.sync.dma_start(wtu[:], w_u[nf])
                    pu = ps.tile([P, QR], dt.float32, tag="mm")
                    for ki in range(KI_H):
                        nc.tensor.matmul(
                            pu[:], wtu[:, ki, :], x2[:, ki, :],
                            start=(ki == 0), stop=(ki == KI_H - 1),
                        )
                    nc.vector.tensor_tensor(msb[:, nf, :], pu[:], gs[:], MUL)

                for nf in range(KI_H):
                    pt = ps.tile([P, QR], dt.float32, tag="mm")
                    for half in range(2):
                        wt = p5.tile([P, NF_FF // 2, P], dt.bfloat16, tag="wd", bufs=2)
                        nc.sync.dma_start(wt[:], w_d[nf, :, half * 32 : (half + 1) * 32, :])
                        for ki in range(NF_FF // 2):
                            kk = half * 32 + ki
                            nc.tensor.matmul(
                                pt[:], wt[:, ki, :], msb[:, kk, :],
                                start=(kk == 0), stop=(kk == NF_FF - 1),
                            )
                    ot = p5.tile([P, QR], dt.float32, tag="h1t", bufs=2)
                    nc.vector.tensor_tensor(ot[:], pt[:], h1sb[:, nf, :], ADD)
                    nc.sync.dma_start(out[nf * P : (nf + 1) * P, :], ot[:])

    return nc


# ---------------------------------------------------------------------------
# host-side packing
# ---------------------------------------------------------------------------
def _deint_perm():
    # deinterleave: out[i] = in[2i] (i<32), in[2(i-32)+1] (i>=32)
    return np.concatenate([np.arange(0, ROPE, 2), np.arange(1, ROPE, 2)])


def _pack_lhst(w, nki, nnf, nfree=P):
    # w [nki*P, nnf*nfree] -> [nnf, P, nki, nfree]
    return np.ascontiguousarray(
        w.reshape(nki, P, nnf, nfree).transpose(2, 1, 0, 3).astype(BF16)
    )


def _fp8(x):
    # TRN FP8_EXP4 matches OCP e4m3 bit patterns for |x| <= 240
    return np.clip(x, -240.0, 240.0).astype(ml_dtypes.float8_e4m3)


def _prep_shared(inputs):
    perm = _deint_perm()
    ln1 = inputs["ln1_w"].astype(np.float32)
    qaln = inputs["q_a_ln_w"].astype(np.float32)
    kvln = inputs["kv_a_ln_w"].astype(np.float32)
    ln2 = inputs["ln2_w"].astype(np.float32)

    w_qa = inputs["q_a_kernel"].astype(np.float32) * ln1[:, None]
    w_kva = inputs["kv_a_kernel"].astype(np.float32) * ln1[:, None]
    w_kva = w_kva.copy()
    # 16x on the rope cols: k_pe is stored fp8 pre-scaled for DoubleRow
    w_kva[:, KV_LORA:] = w_kva[:, KV_LORA:][:, perm] * 16.0
    w_qb = inputs["q_b_kernel"].astype(np.float32) * qaln[:, None]
    w_qb = w_qb.copy()
    for h in range(NH):
        blk = slice(h * QHD + NOPE, (h + 1) * QHD)
        w_qb[:, blk] = w_qb[:, blk][:, perm]
    w_kvb = inputs["kv_b_kernel"].astype(np.float32) * kvln[:, None]
    w_o = inputs["o_kernel"].astype(np.float32)
    w_g = inputs["gate_kernel"].astype(np.float32) * ln2[:, None]
    w_u = inputs["up_kernel"].astype(np.float32) * ln2[:, None]
    w_d = inputs["down_kernel"].astype(np.float32)

    # diagonal-block causal mask, identical on every core:
    # key (kt*128+p) visible to query q  <=>  kt*128+p <= q
    kp = np.arange(P)[:, None]
    qf = np.arange(QR)[None, :]
    mdg = np.zeros((P, TQ, QR), dtype=BF16)
    for kt in range(TQ):
        mdg[:, kt, :] = ((kt * P + kp) <= qf).astype(BF16)

    shared = {
        # DoubleRow fp8: K-tile pairs (2g+i) packed on the middle axes, 16x
        # pre-scaled for fp8 range
        "w_qa": np.ascontiguousarray(
            _fp8(w_qa.reshape(KI_H, P, KI_QL, P).transpose(2, 1, 0, 3) * 16.0)
            .reshape(KI_QL, P, KI_H // 2, 2, P)
        ),
        "w_qb": np.ascontiguousarray(
            _fp8(w_qb.reshape(KI_QL, P, NH, QHD).transpose(2, 1, 0, 3) * 16.0)
            .reshape(NH, P, KI_QL // 2, 2, QHD)
        ),
        # w_kva resident: [P, KI_H, 576]
        "w_kva": np.ascontiguousarray(
            w_kva.reshape(KI_H, P, KV_LORA + ROPE).transpose(1, 0, 2).astype(BF16)
        ),
        # w_kvb split into k/v halves, packed per head-group of 4:
        # [hg, p, lt, hh*128+c]
        # 16x: k_nope lands in fp8 pre-scaled for the DoubleRow score matmul;
        # latent K-tile pairs packed for the DoubleRow k_nope build
        "w_kv_k": np.ascontiguousarray(
            _fp8(
                (w_kvb.reshape(KI_KVL, P, NH // 4, 4, 2, 128)[:, :, :, :, 0, :] * 16.0)
                .transpose(2, 1, 0, 3, 4)
                .reshape(NH // 4, P, KI_KVL, 512)
            ).reshape(NH // 4, P, KI_KVL // 2, 2, 512)
        ),
        "w_kv_v": np.ascontiguousarray(
            w_kvb.reshape(KI_KVL, P, NH // 4, 4, 2, 128)[:, :, :, :, 1, :]
            .transpose(2, 1, 0, 3, 4)
            .reshape(NH // 4, P, KI_KVL, 512)
            .astype(BF16)
        ),
        # w_o: [KI_H(nf), P, NH, VHD]
        "w_o": np.ascontiguousarray(
            w_o.reshape(NH, VHD, KI_H, P).transpose(2, 1, 0, 3).astype(BF16)
        ),
        "w_g": _pack_lhst(w_g, KI_H, NF_FF),
        "w_u": _pack_lhst(w_u, KI_H, NF_FF),
        "w_d": _pack_lhst(w_d, NF_FF, KI_H),
        "maskdiag": mdg,
    }
    return shared


def _prep_batch(inputs, b):
    hid = np.asarray(inputs["hidden_states"][b], dtype=np.float32)  # [S, H]
    hT = np.ascontiguousarray(hid.T)  # [H, S]
    pos = np.asarray(inputs["position_ids"][b]).astype(np.int64)
    cos_g = np.asarray(inputs["cos"], dtype=np.float32)[pos][:, :32]  # [S, 32]
    sin_g = np.asarray(inputs["sin"], dtype=np.float32)[pos][:, :32]
    # [64] rows: cos duplicated; sin with rotate-half signs baked in
    cos2 = np.concatenate([cos_g, cos_g], axis=1)  # [S, 64]
    sin2 = np.concatenate([-sin_g, sin_g], axis=1)
    return hT, np.ascontiguousarray(cos2.T), np.ascontiguousarray(sin2.T)


def _core_colmask(j):
    # multiplicative mask per key tile: block b holds original chunk (j+b)%4.
    # past chunks (< j): 1 (fully visible); future (> j): 0 (masked).
    # block 0 (diagonal) uses the shared banded mask instead.
    cb = np.ones((P, TK), dtype=np.float32)
    for b in range(1, NBLK):
        c = (j + b) % NBLK
        if c > j:
            cb[:, b * TQ : (b + 1) * TQ] = 0.0
    return cb


def kernel(**inputs) -> np.ndarray:
    import concourse.bass as bass  # noqa: F401  (env check)
    from concourse.bass_utils import run_bass_kernel_spmd

    if "nc" not in _COMPILED:
        _COMPILED["nc"] = _build_nc()
    nc = _COMPILED["nc"]

    shared = _prep_shared(inputs)
    in_maps = []
    per_batch = [_prep_batch(inputs, b) for b in range(B)]
    hTb_cache = {}
    for c in range(8):
        b, j = c // 4, c % 4
        hT, cosT, sinT = per_batch[b]
        if b not in hTb_cache:
            hTb_cache[b] = hT.astype(BF16)
        hTbf = hTb_cache[b]
        q0 = j * QR
        rot = [((j + k) % NBLK) for k in range(NBLK)]  # block b -> orig chunk
        in_map = dict(shared)
        # key blocks 1..3 in rotated order (block 0 == the q slice, resident)
        in_map["hTb"] = np.ascontiguousarray(
            np.concatenate([hTbf[:, c_ * QR : (c_ + 1) * QR] for c_ in rot[1:]], axis=1)
        )
        in_map["hTqb"] = np.ascontiguousarray(hTbf[:, q0 : q0 + QR])
        in_map["hTq"] = np.ascontiguousarray(hT[:, q0 : q0 + QR])
        in_map["cosT"] = np.ascontiguousarray(
            np.concatenate([cosT[:, c_ * QR : (c_ + 1) * QR] for c_ in rot], axis=1)
        ).astype(BF16)
        in_map["sinT"] = np.ascontiguousarray(
            np.concatenate([sinT[:, c_ * QR : (c_ + 1) * QR] for c_ in rot], axis=1)
        ).astype(BF16)
        in_map["cosTq"] = np.ascontiguousarray(cosT[:, q0 : q0 + QR])
        in_map["sinTq"] = np.ascontiguousarray(sinT[:, q0 : q0 + QR])
        in_map["colmask"] = _core_colmask(j)
        in_maps.append(in_map)

    res = run_bass_kernel_spmd(nc, in_maps, core_ids=list(range(8)))
    globals()["LAST_RESULT"] = res

    out = np.empty((B, S, H), dtype=np.float32)
    for c in range(8):
        b, j = c // 4, c % 4
        out[b, j * QR : (j + 1) * QR, :] = res.results[c]["out"].T
    return out
